# revision 44
# baseline (speedup 1.0000x reference)
"""Trainium2 Bass kernel for nn_Attention_12137577578573.

Full multi-head attention (QKV projection + masked softmax + context) for
B=4, F=T=2048, CF=CT=1024, H=16, DH=64, sharded over 8 NeuronCores as
(batch b, head-group hg): core i = (b = i // 2, hg = i % 2), each core
computing 1 batch x 8 heads.

v2 schedule (ACT-bound design, ~all stalls removed vs v1):
  - two DMA queues (sync: yT/wk/wv K-path, gpsimd: xT/wq/mask Q-path) so the
    first score tile exists ~10us in instead of ~36us.
  - pT is a fine-grained ring of 32 [128,1024] tiles (2 units) so tile
    lifetime conflicts stall per-tile, not per-unit.
  - context matmuls are emitted column-major (one source-tt "column" = 8
    accumulating MMs across (hh,ft)) with a 2-unit lag: ctx(u) col j right
    before unit u+2 slot j.  The lag ramps down to 0 over units 13-15 so the
    tail after the last ACTIVATE is only ~4us.
  - softmax exp for the last OFF_N[u] tiles of each unit is computed on the
    DVE instead of ACT (Schraudolph exponent bit-trick in bf16, fused with
    the mask multiply), offloading the bottleneck engine.
  - per-tile mask multiplies (DVE 2x) right after each exp tile.

Layout strategy (unchanged from v1):
  - host pre-transposes from/to tensors -> xT/yT [C, F or T] so QKV
    projections contract C on partitions; Q^T/K^T in transposed layout so
    scores contract DH on partitions; 2 heads packed per 128-partition tile
    (concurrent row-group matmuls).  Scores come out S^T [T, F].  Mask folded
    multiplicatively after exp.  Context C = P^T.T @ [V | 1] gives the
    softmax denominator for free; normalized via per-partition reciprocal.
  - reference reshapes K as (T, DH, H); handled by host-side column
    permutation of Wk/bk.
"""

import sys

if "/opt/trn_rl_repo" not in sys.path:
    sys.path.insert(0, "/opt/trn_rl_repo")

import contextlib

import numpy as np
import ml_dtypes

import concourse.bass as bass
import concourse.bacc as bacc
import concourse.mybir as mybir
import concourse.tile as tile
from concourse import bass_utils

BF16 = mybir.dt.bfloat16
F32 = mybir.dt.float32
I16 = mybir.dt.int16
bf16 = ml_dtypes.bfloat16

B, F, T, C, H, DH = 4, 2048, 2048, 1024, 16, 64
HL = 8          # heads per core
COLS = HL * DH  # 512 projected columns per core
ALPHA = 0.125   # 1/sqrt(64)
NCORES = 8
KT = C // 128   # 8 contraction tiles for projections
NFT = F // 128  # 16 F tiles
NTT = T // 128  # 16 T tiles
NU = 16         # units: (fc, pair)

LOG2E = 1.4426950408889634
SCHR_A = ALPHA * LOG2E * 128.0       # score -> bf16-exponent-lattice scale
SCHR_B = 127.0 * 128.0 - 5.0         # exponent bias, mean-error centered

# Schraudolph(DVE)-offloaded exp tile slots per unit.  Mid-unit slots: an
# OFF slot's psum_s buffer is freed by a DVE ts1 op, and end-of-unit ts1s
# drain late (after all the unit's masks), stalling the next unit's scores.
# Units 0-3 are PE-bound (projection fillers), offload doesn't help there.
OFF_SLOTS = [()] * 4 + [(5, 11)] * 12

PROFILE = False
LAST_RESULTS = None

_nc_cache = None


def _emit(tc, nc, aps):
    xT, yT, maskT, wq, wk, wv, bq, bk, bv, out = aps
    Exp = mybir.ActivationFunctionType.Exp
    Mult = mybir.AluOpType.mult
    Add = mybir.AluOpType.add

    with contextlib.ExitStack() as ctx:
        pool = ctx.enter_context(tc.tile_pool(name="static", bufs=1))
        xTp = ctx.enter_context(tc.tile_pool(name="xTp", bufs=2))
        qTp = ctx.enter_context(tc.tile_pool(name="qTp", bufs=2))
        maskp = ctx.enter_context(tc.tile_pool(name="maskp", bufs=3))
        pTp = ctx.enter_context(tc.tile_pool(name="pTp", bufs=32))
        outp = ctx.enter_context(tc.tile_pool(name="outp", bufs=1))
        schrp = ctx.enter_context(tc.tile_pool(name="schrp", bufs=1))
        psum_s = ctx.enter_context(tc.tile_pool(name="psum_s", bufs=3, space="PSUM"))
        psum_ctx = ctx.enter_context(tc.tile_pool(name="psum_ctx", bufs=2, space="PSUM"))

        # ---- static tiles ----
        yT_sb = pool.tile([128, KT, T], BF16, name="yT_sb", tag="yT_sb")
        wq_sb = pool.tile([128, KT, COLS], BF16, name="wq_sb", tag="wq_sb")
        wk_sb = pool.tile([128, KT, COLS], BF16, name="wk_sb", tag="wk_sb")
        wv_sb = pool.tile([128, KT, COLS], BF16, name="wv_sb", tag="wv_sb")
        kT = [pool.tile([128, T], BF16, name=f"kT{cb}", tag=f"kT{cb}") for cb in range(4)]
        v = [pool.tile([128, HL * 65], BF16, name=f"v{tt}", tag=f"v{tt}") for tt in range(NTT)]
        bq_sb = pool.tile([128, 4], F32, name="bq_sb", tag="bq_sb")
        bk_sb = pool.tile([128, 4], F32, name="bk_sb", tag="bk_sb")
        bv_sb = pool.tile([1, COLS], BF16, name="bv_sb", tag="bv_sb")
        ones_sb = pool.tile([1, 128], BF16, name="ones_sb", tag="ones_sb")

        xT_r = xT.rearrange("(k p) f -> p k f", p=128)
        yT_r = yT.rearrange("(k p) t -> p k t", p=128)
        wq_r = wq.rearrange("(k p) c -> p k c", p=128)
        wk_r = wk.rearrange("(k p) c -> p k c", p=128)
        wv_r = wv.rearrange("(k p) c -> p k c", p=128)
        maskT_r = maskT.rearrange("(tt p) f -> p tt f", p=128)
        out_r = out.rearrange("(g p) c -> p g c", p=128)

        # ---- upfront work: two HWDGE DMA rings + engine warmups ----
        # gpsimd (SWDGE, idle): tiny bias loads
        nc.gpsimd.dma_start(bk_sb[:], bk[:])
        nc.gpsimd.dma_start(bq_sb[:], bq[:])
        nc.gpsimd.dma_start(bv_sb[:], bv[:])
        nc.vector.memset(ones_sb[:], 1.0)
        warm_sb = pool.tile([1, 8], F32, name="warm_sb", tag="warm_sb")
        nc.vector.memset(warm_sb[:], 0.0)
        nc.scalar.activation(warm_sb[:], warm_sb[:], Exp)
        # ones columns of all V tiles, written once
        for tt in range(NTT):
            vview = v[tt].rearrange("p (h c) -> p h c", c=65)
            nc.vector.memset(vview[:, :, 64:65], 1.0)

        # sync ring: K-projection path, in exact first-use order (512KB chunks
        # so the first score tile's inputs aren't queued behind later data)
        nc.sync.dma_start(yT_sb[:, :, 0:256], yT_r[:, :, 0:256])
        nc.sync.dma_start(wk_sb[:, :, 0:256], wk_r[:, :, 0:256])
        nc.sync.dma_start(yT_sb[:, :, 256:512], yT_r[:, :, 256:512])
        nc.sync.dma_start(wk_sb[:, :, 256:512], wk_r[:, :, 256:512])
        nc.sync.dma_start(yT_sb[:, :, 512:1024], yT_r[:, :, 512:1024])
        nc.sync.dma_start(yT_sb[:, :, 1024:1536], yT_r[:, :, 1024:1536])
        nc.sync.dma_start(yT_sb[:, :, 1536:2048], yT_r[:, :, 1536:2048])

        # scalar ring (2nd HWDGE, idle in prologue): Q path + first masks +
        # a later yT quarter (ring load balance)
        xT_tiles = {}
        xT_tiles[0] = xTp.tile([128, KT, 512], BF16, name="xTt", tag="xT")
        nc.scalar.dma_start(xT_tiles[0][:], xT_r[:, :, 0:512])
        nc.scalar.dma_start(wq_sb[:, :, 0:256], wq_r[:, :, 0:256])
        mask_h = {}
        mask_h[(0, 0)] = maskp.tile([128, 8, 512], BF16, name="mh", tag="mask")
        nc.scalar.dma_start(mask_h[(0, 0)][:], maskT_r[:, 0:8, 0:512])
        nc.scalar.dma_start(wq_sb[:, :, 256:512], wq_r[:, :, 256:512])
        nc.scalar.dma_start(wv_sb[:, :, 0:256], wv_r[:, :, 0:256])
        nc.scalar.dma_start(wv_sb[:, :, 256:512], wv_r[:, :, 256:512])
        mask_h[(0, 1)] = maskp.tile([128, 8, 512], BF16, name="mh", tag="mask")
        nc.scalar.dma_start(mask_h[(0, 1)][:], maskT_r[:, 8:16, 0:512])

        def dma_xt(fc):
            def go():
                xt = xTp.tile([128, KT, 512], BF16, name="xTt", tag="xT")
                nc.sync.dma_start(xt[:], xT_r[:, :, fc * 512:(fc + 1) * 512])
                xT_tiles[fc] = xt
            return go

        def dma_mask(fc, half):
            def go():
                mh = maskp.tile([128, 8, 512], BF16, name="mh", tag="mask")
                nc.sync.dma_start(
                    mh[:],
                    maskT_r[:, half * 8:(half + 1) * 8, fc * 512:(fc + 1) * 512],
                )
                mask_h[(fc, half)] = mh
            return go

        # ---- projection chains as 5-step closures (4 MM pairs + evac) ----
        qT_tiles = {}

        def k_chain_steps(cb, tcc):
            st = {}
            def half(i):
                if i == 0:
                    st["ps"] = psum_s.tile([128, 1024], F32, name="ps_f", tag="s")[:, 0:512]
                ps = st["ps"]
                for k in range(4 * i, 4 * i + 4):
                    nc.tensor.matmul(
                        ps[:],
                        wk_sb[:, k, cb * 128:(cb + 1) * 128],
                        yT_sb[:, k, tcc * 512:(tcc + 1) * 512],
                        start=(k == 0),
                        stop=(k == KT - 1),
                    )
                if i == 1:
                    nc.vector.tensor_scalar_add(
                        kT[cb][:, tcc * 512:(tcc + 1) * 512], ps, bk_sb[:, cb:cb + 1]
                    )
            return [lambda i=i: half(i) for i in range(2)]

        def q_chain_steps(fc, cb):
            st = {}
            def half(i):
                if i == 0:
                    if fc not in qT_tiles:
                        qT_tiles[fc] = qTp.tile([128, 4, 512], BF16, name="qTt", tag="qT")
                    st["ps"] = psum_s.tile([128, 1024], F32, name="ps_f", tag="s")[:, 0:512]
                ps = st["ps"]
                xt = xT_tiles[fc]
                for k in range(4 * i, 4 * i + 4):
                    nc.tensor.matmul(
                        ps[:],
                        wq_sb[:, k, cb * 128:(cb + 1) * 128],
                        xt[:, k, :],
                        start=(k == 0),
                        stop=(k == KT - 1),
                    )
                if i == 1:
                    nc.vector.tensor_scalar_add(
                        qT_tiles[fc][:, cb, :], ps, bq_sb[:, cb:cb + 1]
                    )
            return [lambda i=i: half(i) for i in range(2)]

        def v_chain_steps(tt, vh):
            # half-width V chain: head columns vh*256:(vh+1)*256 = pairs
            # 2vh, 2vh+1 — lets pairs 0-1's V (needed by ctx(0..1)) land
            # early while pairs 2-3's V waits for later units
            st = {}
            def half(i):
                if i == 0:
                    st["ps"] = psum_s.tile([128, 1024], F32, name="ps_f", tag="s")[:, 0:256]
                ps = st["ps"]
                for k in range(4 * i, 4 * i + 4):
                    nc.tensor.matmul(
                        ps[:],
                        yT_sb[:, k, tt * 128:(tt + 1) * 128],
                        wv_sb[:, k, vh * 256:(vh + 1) * 256],
                        start=(k == 0),
                        stop=False,
                    )
                if i == 1:
                    nc.tensor.matmul(
                        ps[:], ones_sb[0:1, :], bv_sb[0:1, vh * 256:(vh + 1) * 256],
                        start=False, stop=True,
                    )
                    vview = v[tt].rearrange("p (h c) -> p h c", c=65)
                    nc.vector.tensor_copy(
                        vview[:, vh * 4:(vh + 1) * 4, 0:64],
                        ps.rearrange("p (h c) -> p h c", c=64)[:],
                    )
            return [lambda i=i: half(i) for i in range(2)]

        # ---- scores + exp + mask ----
        score_ps = {}
        pT_t = {}

        def emit_score(u, tt):
            fc, pair = u // 4, u % 4
            ps = psum_s.tile([128, 1024], F32, name="ps_s", tag="s")
            qt = qT_tiles[fc]
            for hh in range(2):
                nc.tensor.matmul(
                    ps[:, hh * 512:(hh + 1) * 512],
                    kT[pair][hh * 64:(hh + 1) * 64, tt * 128:(tt + 1) * 128],
                    qt[hh * 64:(hh + 1) * 64, pair, :],
                    start=True,
                    stop=True,
                )
            score_ps[(u, tt)] = ps

        def emit_exp_mask(u, tt):
            fc = u // 4
            offload = tt in OFF_SLOTS[u]
            ps = score_ps.pop((u, tt))
            pt = pTp.tile([128, 1024], BF16, name="pT", tag="pT")
            pT_t[(u, tt)] = pt
            mh = mask_h[(fc, tt // 8)]
            m = mh[:, tt % 8:tt % 8 + 1, :].broadcast_to([128, 2, 512])
            o = pt.rearrange("p (h c) -> p h c", c=512)
            if not offload:
                nc.scalar.activation(pt[:], ps[:], Exp, scale=ALPHA)
                nc.vector.tensor_mul(o[:], o[:], m)
            else:
                it = schrp.tile([128, 1024], I16, name="schr", tag="schr")
                nc.vector.tensor_scalar(
                    it[:], ps[:], SCHR_A, SCHR_B, op0=Mult, op1=Add
                )
                nc.vector.tensor_mul(
                    o[:],
                    it[:].bitcast(BF16).rearrange("p (h c) -> p h c", c=512),
                    m,
                )

        # ---- context: column-major accumulation ----
        ctx_ps = {}

        def emit_ctx_col(cu, c, first, last):
            pair = cu % 4
            if c == first:
                ctx_ps[cu] = {
                    hh: psum_ctx.tile([128, 512], F32, name="pc", tag="pc")
                    for hh in range(2)
                }
            pt = pT_t[(cu, c)]
            for hh in range(2):
                pc = ctx_ps[cu][hh]
                h = pair * 2 + hh
                for ft in range(4):
                    # start clears the whole PSUM bank's has_written bits, so
                    # it must be set on the bank's FIRST matmul only (ft==0);
                    # ft 1-3 of the first column land on cleared flags and
                    # overwrite, later columns accumulate.
                    nc.tensor.matmul(
                        pc[:, ft * 65:ft * 65 + 65],
                        pt[:, hh * 512 + ft * 128:hh * 512 + (ft + 1) * 128],
                        v[c][:, h * 65:(h + 1) * 65],
                        start=(c == first and ft == 0),
                        stop=(c == last and ft == 3),
                        skip_group_check=True,
                    )

        def emit_ctx_evac(cu):
            # raw numerators + denominators out; normalization on host
            fc, pair = cu // 4, cu % 4
            ot = outp.tile([128, 4, 130], F32, name="outt", tag="out")
            for hh in range(2):
                pc = ctx_ps[cu][hh]
                nc.vector.tensor_copy(
                    ot[:, :, hh * 65:(hh + 1) * 65],
                    pc[:, 0:260].rearrange("p (ft c) -> p ft c", c=65)[:],
                )
            ctx_ps.pop(cu)
            for tt in range(NTT):
                pT_t.pop((cu, tt), None)
            nc.gpsimd.dma_start(
                out_r[:, fc * 4:(fc + 1) * 4, pair * 130:(pair + 1) * 130],
                ot[:],
            )

        # ---- schedules ----
        # ctx columns emitted at (unit, slot): list of (cu, col, first, last)
        ctx_sched = {}

        def add_ctx(w, s, cu, c, first=0, last=15):
            ctx_sched.setdefault((w, s), []).append((cu, c, first, last))

        for w in range(1, 15):          # 1-unit lag steady state
            for s in range(NTT):
                add_ctx(w, s, w - 1, s)
        # tail compression: ctx(14)'s sources are all ready at u15 start, so
        # run it 2 cols/slot in u15's first half; evac it at slot 8, freeing
        # the psum_ctx buffers for ctx(15) to run 2 cols/slot in the second
        # half (only cols 14-15 + evac remain after the loop)
        for s in range(8):
            add_ctx(15, s, 14, 2 * s)
            add_ctx(15, s, 14, 2 * s + 1)
        for c in range(14):
            add_ctx(15, 9 + c // 2, 15, c)

        # evac at (unit, slot) — must come after the cu's last (stop) column
        evac_sched = {}
        for cu in range(14):
            evac_sched.setdefault((cu + 1, 15), []).append(cu)
        evac_sched.setdefault((15, 8), []).append(14)

        # filler chains, slot-addressed.  Hard ordering constraints (PE FIFO:
        # a waiting instruction must never depend on later PE work):
        #   k(cb,tcc) fully emitted before unit cb slot 4*tcc's score
        #   q(fc,cb) fully emitted before unit 4*fc+cb slot 0
        #   v(c) fully emitted before ctx(0) col c at unit 2 slot c
        fillers = {}

        def place(u, chains, per_slot=2):
            steps = [st for ch in chains for st in ch]
            for j, st in enumerate(steps):
                fillers.setdefault((u, j // per_slot), []).append(st)

        place(0, [k_chain_steps(1, 0), k_chain_steps(1, 1), k_chain_steps(0, 1),
                  k_chain_steps(1, 2), k_chain_steps(1, 3), k_chain_steps(0, 2),
                  q_chain_steps(0, 1), k_chain_steps(0, 3), v_chain_steps(0, 0),
                  v_chain_steps(1, 0), v_chain_steps(2, 0), v_chain_steps(3, 0),
                  v_chain_steps(4, 0), v_chain_steps(5, 0)])
        place(1, [v_chain_steps(6, 0), v_chain_steps(7, 0), k_chain_steps(2, 0),
                  v_chain_steps(8, 0), k_chain_steps(2, 1), v_chain_steps(9, 0),
                  q_chain_steps(0, 2), v_chain_steps(10, 0), v_chain_steps(11, 0),
                  v_chain_steps(12, 0), v_chain_steps(13, 0), v_chain_steps(14, 0),
                  v_chain_steps(15, 0)])
        place(2, [k_chain_steps(2, 2), k_chain_steps(2, 3), k_chain_steps(3, 0),
                  k_chain_steps(3, 1), q_chain_steps(0, 3), v_chain_steps(0, 1),
                  v_chain_steps(1, 1), v_chain_steps(2, 1), v_chain_steps(3, 1),
                  v_chain_steps(4, 1), v_chain_steps(5, 1)])
        place(3, [v_chain_steps(6, 1), k_chain_steps(3, 2), v_chain_steps(7, 1),
                  k_chain_steps(3, 3), v_chain_steps(8, 1), v_chain_steps(9, 1),
                  v_chain_steps(10, 1), v_chain_steps(11, 1), v_chain_steps(12, 1),
                  v_chain_steps(13, 1), v_chain_steps(14, 1), v_chain_steps(15, 1),
                  q_chain_steps(1, 0)])
        place(4, [q_chain_steps(1, 1)])
        place(5, [q_chain_steps(1, 2)], per_slot=1)
        place(6, [q_chain_steps(1, 3)], per_slot=1)
        place(7, [q_chain_steps(2, 0)], per_slot=1)
        place(8, [q_chain_steps(2, 1)], per_slot=1)
        place(9, [q_chain_steps(2, 2)], per_slot=1)
        place(10, [q_chain_steps(2, 3)], per_slot=1)
        place(11, [q_chain_steps(3, 0)], per_slot=1)
        place(12, [q_chain_steps(3, 1)], per_slot=1)
        place(13, [q_chain_steps(3, 2)], per_slot=1)
        place(14, [q_chain_steps(3, 3)], per_slot=1)

        # mid-stream DMA triggers at (unit, slot); deferred bulk loads first
        def dma_sync(dst, src):
            return lambda: nc.sync.dma_start(dst, src)

        dma_sched = {
            (1, 2): [dma_xt(1)],
            (2, 2): [dma_mask(1, 0)],
            (3, 2): [dma_mask(1, 1)],
            (5, 2): [dma_xt(2)],
            (6, 2): [dma_mask(2, 0)],
            (7, 2): [dma_mask(2, 1)],
            (9, 2): [dma_xt(3)],
            (10, 2): [dma_mask(3, 0)],
            (11, 2): [dma_mask(3, 1)],
        }

        # ---- prologue compute: k(0,0) in T-eighths so the first score tile
        # only needs the first 512KB of yT/wk ----
        def k8_chain(tc8):
            ps = psum_s.tile([128, 1024], F32, name="ps_f", tag="s")[:, 0:256]
            for k in range(KT):
                nc.tensor.matmul(
                    ps[:],
                    wk_sb[:, k, 0:128],
                    yT_sb[:, k, tc8 * 256:(tc8 + 1) * 256],
                    start=(k == 0),
                    stop=(k == KT - 1),
                )
            nc.vector.tensor_scalar_add(
                kT[0][:, tc8 * 256:(tc8 + 1) * 256], ps, bk_sb[:, 0:1]
            )

        k8_chain(0)
        for st in q_chain_steps(0, 0):
            st()
        k8_chain(1)

        # ---- main unit loop ----
        for u in range(NU):
            for s in range(NTT):
                for d in dma_sched.get((u, s), ()):
                    d()
                # ctx columns first: they unblock this slot's pT ring alloc
                for (cu, c, first, last) in ctx_sched.get((u, s), ()):
                    emit_ctx_col(cu, c, first, last)
                emit_score(u, s)
                emit_exp_mask(u, s)
                for st in fillers.get((u, s), ()):
                    st()
                for cu in evac_sched.get((u, s), ()):
                    emit_ctx_evac(cu)
        # tail: last two ctx(15) columns + its evacuation
        for c in (14, 15):
            emit_ctx_col(15, c, 0, 15)
        emit_ctx_evac(15)


def _build():
    global _nc_cache
    if _nc_cache is not None:
        return _nc_cache
    nc = bacc.Bacc(
        "TRN2",
        target_bir_lowering=False,
        debug=False,
        enable_asserts=False,
        num_devices=NCORES,
    )
    xT = nc.dram_tensor("xT", [C, F], BF16, kind="ExternalInput").ap()
    yT = nc.dram_tensor("yT", [C, T], BF16, kind="ExternalInput").ap()
    maskT = nc.dram_tensor("maskT", [T, F], BF16, kind="ExternalInput").ap()
    wq = nc.dram_tensor("wq", [C, COLS], BF16, kind="ExternalInput").ap()
    wk = nc.dram_tensor("wk", [C, COLS], BF16, kind="ExternalInput").ap()
    wv = nc.dram_tensor("wv", [C, COLS], BF16, kind="ExternalInput").ap()
    bq = nc.dram_tensor("bq", [128, 4], F32, kind="ExternalInput").ap()
    bk = nc.dram_tensor("bk", [128, 4], F32, kind="ExternalInput").ap()
    bv = nc.dram_tensor("bv", [1, COLS], BF16, kind="ExternalInput").ap()
    out = nc.dram_tensor("out", [F, 4 * 130], F32, kind="ExternalOutput").ap()

    with tile.TileContext(nc) as tc:
        _emit(tc, nc, (xT, yT, maskT, wq, wk, wv, bq, bk, bv, out))
    nc.compile()
    _nc_cache = nc
    return nc


def _kperm(hg):
    """Local K column (pair*128 + hh*64 + d) -> global Wk column d*H + h_g."""
    idx = np.empty(COLS, dtype=np.int64)
    for pair in range(4):
        for hh in range(2):
            h_g = hg * HL + pair * 2 + hh
            for d in range(DH):
                idx[pair * 128 + hh * 64 + d] = d * H + h_g
    return idx


def make_in_maps(from_tensor, to_tensor, mask, Wq, bq, Wk, bk, Wv, bv):
    per_b = {}
    for b in range(B):
        per_b[b] = (
            np.ascontiguousarray(from_tensor[b].T).astype(bf16),
            np.ascontiguousarray(to_tensor[b].T).astype(bf16),
            np.ascontiguousarray(mask[b].T).astype(bf16),
        )
    in_maps = []
    for i in range(NCORES):
        b, hg = i // 2, i % 2
        xTb, yTb, mTb = per_b[b]
        sl = slice(hg * COLS, (hg + 1) * COLS)
        kidx = _kperm(hg)
        in_maps.append(
            {
                "xT": xTb,
                "yT": yTb,
                "maskT": mTb,
                "wq": np.ascontiguousarray(Wq[:, sl]).astype(bf16),
                "wk": np.ascontiguousarray(Wk[:, kidx]).astype(bf16),
                "wv": np.ascontiguousarray(Wv[:, sl]).astype(bf16),
                "bq": np.ascontiguousarray(
                    bq[sl].astype(np.float32).reshape(4, 128).T
                ),
                "bk": np.ascontiguousarray(
                    bk[kidx].astype(np.float32).reshape(4, 128).T
                ),
                "bv": bv[sl].astype(bf16).reshape(1, COLS),
            }
        )
    return in_maps


def kernel(from_tensor, to_tensor, mask, Wq, bq, Wk, bk, Wv, bv):
    global LAST_RESULTS
    from_tensor = np.asarray(from_tensor, dtype=np.float32)
    to_tensor = np.asarray(to_tensor, dtype=np.float32)
    mask_np = np.asarray(mask)
    Wq = np.asarray(Wq, dtype=np.float32)
    Wk = np.asarray(Wk, dtype=np.float32)
    Wv = np.asarray(Wv, dtype=np.float32)
    bq = np.asarray(bq, dtype=np.float32)
    bk = np.asarray(bk, dtype=np.float32)
    bv = np.asarray(bv, dtype=np.float32)

    nc = _build()
    in_maps = make_in_maps(
        from_tensor, to_tensor, mask_np, Wq, bq, Wk, bk, Wv, bv
    )
    res = bass_utils.run_bass_kernel_spmd(
        nc, in_maps, core_ids=list(range(NCORES)), trace=PROFILE
    )
    LAST_RESULTS = res
    full = np.empty((B, F, H * DH), np.float32)
    for i in range(NCORES):
        b, hg = i // 2, i % 2
        o = res.results[i]["out"].reshape(F, 4, 2, 65)
        ctxv = o[..., :64] / o[..., 64:65]
        full[b, :, hg * COLS:(hg + 1) * COLS] = ctxv.reshape(F, COLS)
    return full
